# revision 17
# baseline (speedup 1.0000x reference)
"""ArcFace margin loss kernel for 8 TRN2 NeuronCores.

out = S * logits everywhere except at (i, labels[i]) where
out = S * cos(arccos(x) + m) = S*(x*cos(m) - sqrt(1-x^2)*sin(m)).

Sharding: logits [B=256, C=100000] split along C into 8 shards of
[256, 12500] (Partial-FC style), each viewed flat as [128, 25000]
(2 rows per partition).

The bulk stream is staged in bf16: the op is a pure x64 scale whose
output tolerance (2e-2) is 10x looser than bf16 rounding (2^-9), so the
host downcasts each shard to bf16 and the device streams bf16 in/out,
halving HBM traffic vs fp32 (the memory roofline for this kernel).
x64 is an exact exponent shift in bf16, so the bulk path adds no error
beyond the initial rounding.

Margin handling exploits that the op is elementwise and therefore
permutation-equivariant: the host stores each row's class columns
ROTATED so that the row's target class (if it falls in this shard)
sits at rotated column JSTAR. Every target then lives at flat column
JSTAR (even rows) or 12500+JSTAR (odd rows) of the [128, 25000] view —
two fixed single columns. The device computes the pre-scale margin
blend y = A*x - B*sin(theta) in fp32 on the Vector engine (A/B fold
cos/sin(m) and the in-shard mask; x and sin(theta)=sqrt(1-x^2) are
host-packed fp32 per row — bf16 targets would lose too much precision
through the arccos derivative) and pokes the two values into the
covering bulk tiles IN SBUF between load and scale — no indirect DMA,
no extra stores, no ordering against the bulk stream, and the x64
scale then applies S to the fixed columns along with everything else.
JSTAR is chosen so the two strip columns land in mid-pipeline tiles,
giving the fixup chain several microseconds of slack. The host
un-rotates the output rows when assembling the result.
"""

import numpy as np
import ml_dtypes

S = 64.0
MARGIN = 0.5
B, C, M = 256, 100000, 8
CS = C // M            # 12500 classes per core
P = 128                # SBUF partitions
FREE = (B * CS) // P   # 25000 flat elements per partition
# bulk column tiles over [0, 25000): small first tiles so the
# load->scale->store pipeline ramps fast (the first store is gated on
# the first load completing, and DMA completion carries a ~2us receipt
# latency), balanced middle, small last tiles so the final
# load->scale->store chain isn't serialized behind a large transfer.
TILE_COLS = [1024, 2048, 4096, 4458, 4458, 4458, 3408, 1050]
assert sum(TILE_COLS) == FREE
NT = len(TILE_COLS)
TILE_OFF = np.cumsum([0] + TILE_COLS).tolist()
JSTAR = 8000           # rotated target column; strip cols JSTAR, CS+JSTAR
STRIP = []             # (tile index, local col) for the two strip columns
for _c in (JSTAR, CS + JSTAR):
    _k = next(i for i in range(NT) if TILE_OFF[i] <= _c < TILE_OFF[i + 1])
    STRIP.append((_k, _c - TILE_OFF[_k]))
assert STRIP[0][0] != STRIP[1][0] and all(k < NT - 1 for k, _ in STRIP)

_graph_cache = {}


def _build_graph():
    import concourse.bacc as bacc
    import concourse.tile as tile
    from concourse import mybir

    f32 = mybir.dt.float32
    bf16 = mybir.dt.bfloat16

    nc = bacc.Bacc()
    logits = nc.declare_dram_parameter("logits", [P, FREE], bf16, isOutput=False)
    small = nc.declare_dram_parameter("small", [P, 8], f32, isOutput=False)
    out = nc.declare_dram_parameter("out", [P, FREE], bf16, isOutput=True)

    with tile.TileContext(nc) as tc:
        with (
            tc.tile_pool(name="bulk", bufs=NT) as pool,
            tc.tile_pool(name="fix", bufs=1) as fix,
        ):
            # ---- margin strip values. small = [x_e, x_o, sin_e, sin_o,
            # A_e, A_o, B_e, B_o] per partition (even/odd row of the
            # pair); y = A*x - B*sin in fp32. Loaded on the Scalar HWDGE
            # ring (idle until the first bulk store ~4us later) and
            # computed on the Vector engine ahead of the bulk scales.
            # GpSimd/Pool stays completely unused: its SWDGE path adds
            # Q7 ucode library loads and a multi-us engine drain to the
            # epilogue, all inside the measured window.
            sm_t = fix.tile([P, 8], f32)
            nc.scalar.dma_start(sm_t[:], small[:])
            ya_t = fix.tile([P, 2], f32)
            nc.vector.tensor_mul(ya_t[:], sm_t[:, 0:2], sm_t[:, 4:6])
            yb_t = fix.tile([P, 2], f32)
            nc.vector.tensor_mul(yb_t[:], sm_t[:, 2:4], sm_t[:, 6:8])
            y_t = fix.tile([P, 2], bf16)
            nc.vector.tensor_sub(y_t[:], ya_t[:], yb_t[:])

            # ---- bulk x64 scale, streamed in NT bf16 column tiles.
            # Loads issue from the Sync HWDGE ring, stores from the Scalar
            # (Activation) HWDGE ring, scale on the Vector engine — three
            # independent issue streams, one SBUF slot per tile. HWDGE
            # ring descriptors drain FIFO, so completion order = issue
            # order. ALL loads are emitted before any store: DMA
            # completion-semaphore lanes are reused round-robin with
            # period 8, so this order pairs store_k's lane with load_k —
            # a dependency store_k already has transitively — instead of
            # gating late LOAD issues on mid-stream store completions
            # (which starves the load stream and idles the DMA engines).
            # The two strip tiles get their target column poked in SBUF
            # between load and scale.
            tiles = []
            for k, f in enumerate(TILE_COLS):
                o = TILE_OFF[k]
                bt = pool.tile([P, f], bf16)
                nc.sync.dma_start(bt[:], logits[:, o : o + f])
                tiles.append(bt)
            for k, f in enumerate(TILE_COLS):
                o = TILE_OFF[k]
                bt = tiles[k]
                for si, (sk, sc) in enumerate(STRIP):
                    if sk == k:
                        # poke the pre-scale margin value into the tile's
                        # target column; on the Vector engine right before
                        # the tile's own scale, so in-order execution
                        # guarantees poke -> scale -> store
                        nc.vector.tensor_scalar_mul(
                            bt[:, sc : sc + 1], y_t[:, si : si + 1], 1.0
                        )
                nc.vector.tensor_scalar_mul(bt[:], bt[:], S)
                nc.scalar.dma_start(out[:, o : o + f], bt[:])
    nc.finalize()
    return nc


def _get_graph():
    if "nc" not in _graph_cache:
        _graph_cache["nc"] = _build_graph()
    return _graph_cache["nc"]


def _make_in_maps(logits, labels):
    logits = np.asarray(logits, dtype=np.float32)
    labels = np.asarray(labels).astype(np.int64)
    valid = labels != -1
    rows = np.arange(B)
    cos_m, sin_m = float(np.cos(MARGIN)), float(np.sin(MARGIN))
    cols = np.arange(CS)[None, :]

    in_maps = []
    shifts = []
    for m in range(M):
        shard = logits[:, m * CS : (m + 1) * CS]
        l_loc = labels - m * CS
        in_shard = valid & (l_loc >= 0) & (l_loc < CS)
        # rotate each row so its target class sits at rotated column
        # JSTAR; the elementwise op commutes with the permutation and
        # the host un-rotates the output
        s = np.where(in_shard, (l_loc - JSTAR) % CS, 0).astype(np.int64)
        idx = (cols + s[:, None]) % CS
        rot_bf = np.take_along_axis(
            shard.astype(ml_dtypes.bfloat16), idx, axis=1
        ).reshape(P, FREE)

        x = shard[rows, (JSTAR + s) % CS]           # fp32 strip values
        sin = np.where(
            in_shard, np.sqrt(np.maximum(0.0, 1.0 - x.astype(np.float64) ** 2)), 0.0
        ).astype(np.float32)
        a = np.where(in_shard, cos_m, 1.0).astype(np.float32)
        b = np.where(in_shard, sin_m, 0.0).astype(np.float32)
        # interleave even/odd rows of each partition pair: [P, 8]
        sm = np.stack(
            [
                x[0::2], x[1::2],
                sin[0::2], sin[1::2],
                a[0::2], a[1::2],
                b[0::2], b[1::2],
            ],
            axis=1,
        ).astype(np.float32)
        in_maps.append({"logits": rot_bf, "small": sm})
        shifts.append(s)
    return in_maps, shifts


def _assemble(results, shifts):
    """Un-rotate each core's output shard and concatenate to [B, C]."""
    cols = np.arange(CS)[None, :]
    shards = []
    for m in range(M):
        rot = np.asarray(results[m]["out"]).astype(np.float32).reshape(B, CS)
        idx = (cols + shifts[m][:, None]) % CS
        unrot = np.empty_like(rot)
        np.put_along_axis(unrot, idx, rot, axis=1)
        shards.append(unrot)
    return np.concatenate(shards, axis=1)


def kernel(logits, labels):
    from concourse.bass_utils import run_bass_kernel_spmd

    nc = _get_graph()
    in_maps, shifts = _make_in_maps(np.asarray(logits), labels)
    res = run_bass_kernel_spmd(nc, in_maps, core_ids=list(range(M)))
    return _assemble(res.results, shifts)
